# revision 1
# baseline (speedup 1.0000x reference)
"""Trainium2 Bass kernel for relative-position attention (nn_Attention_14714557956326).

Full inputs:
  x       [4, 1024, 1024] f32
  Wq      [1024, 1024]    f32   (dim -> 16 heads * 64)
  Wkv     [1024, 2048]    f32   (k cols 0..1023, v cols 1024..2047)
  pos_emb [1025, 64]      f32
Output: [4, 16, 1024, 64] f32  (softmax(q k^T * s + rel_pos_bias) v, per head)

Sharding: 8 cores; core c handles batch c//2, heads 8*(c%2) .. +8.
Everything else is per-core local (no collectives).
"""

import sys

sys.path.insert(0, "/opt/trn_rl_repo")

import numpy as np
from contextlib import ExitStack

import concourse.bass as bass
import concourse.bacc as bacc
import concourse.tile as tile
from concourse import mybir
from concourse.ap import AP
from concourse.bass_utils import run_bass_kernel_spmd

# ---------------- problem constants ----------------
B = 4
N_HEADS_TOT = 16
D = 64
DIM = 1024
SEQ = 1024
MAX_POS = 512
TABLE = 2 * MAX_POS + 1  # 1025
SCALE = D ** -0.5

NC = 8              # cores
NH = 8              # heads per core
HD = NH * D         # 512 projected cols per matrix per core
KC = DIM // 128     # 8 contraction chunks
IT = SEQ // 128     # 8 row tiles
RW = 1024           # on-chip table cols c=0..1023; c=1024 via W* projection
EXT = 2 * TABLE - 3  # 2047 ext width
C_SHIFT = 8.0       # exp(x - C) to keep fp16 P in range

F32 = mybir.dt.float32
F32R = mybir.dt.float32r
F16 = mybir.dt.float16

_cached = {}


def build_nc(seq=SEQ, nh=NH, bench_iters=1, ablate=()):
    ablate = set(ablate)
    """Build the per-core Bass program (SPMD: same program on all 8 cores)."""
    it = seq // 128
    hd = nh * D

    nc = bacc.Bacc(
        "TRN2",
        target_bir_lowering=False,
        debug=False,
        enable_asserts=False,
        num_devices=NC,
    )

    xT = nc.dram_tensor("xT", [DIM, seq], F32, kind="ExternalInput")
    Wq = nc.dram_tensor("Wq", [DIM, hd], F32, kind="ExternalInput")
    Wk = nc.dram_tensor("Wk", [DIM, hd], F32, kind="ExternalInput")
    Wv = nc.dram_tensor("Wv", [DIM, hd + nh], F32, kind="ExternalInput")
    TrevT = nc.dram_tensor("TrevT", [D, RW], F32, kind="ExternalInput")
    out = nc.dram_tensor("out", [nh, D, seq], F32, kind="ExternalOutput")

    with tile.TileContext(nc) as tc, ExitStack() as ctx:
        if bench_iters > 1:
            ctx.enter_context(
                tc.For_i(
                    0, bench_iters, 1,
                    hint_engines=(
                        mybir.EngineType.PE,
                        mybir.EngineType.DVE,
                        mybir.EngineType.Activation,
                        mybir.EngineType.SP,
                        mybir.EngineType.Pool,
                    ),
                    name="bench",
                )
            )
        # ---------------- persistent pools ----------------
        const_pool = ctx.enter_context(tc.tile_pool(name="const", bufs=1))
        qk_pool = ctx.enter_context(tc.tile_pool(name="qk", bufs=1))
        v_pool = ctx.enter_context(tc.tile_pool(name="v", bufs=1))

        # pos table, duplicated rows 0-63 and 64-127 so odd heads can slice base 64
        trev_dup = const_pool.tile([128, RW], F32R, tag="trevdup")
        exp_bias = const_pool.tile([128, 1], F32, tag="expbias")
        nc.gpsimd.memset(exp_bias[:], -C_SHIFT)

        qT = [const_pool.tile([128, seq], F32R, name=f"qT{m}", tag=f"qT{m}") for m in range(nh // 2)]
        kT = [qk_pool.tile([128, seq], F32R, name=f"kT{m}", tag=f"kT{m}") for m in range(nh // 2)]
        V = [v_pool.tile([128, nh * D], F16, name=f"V{j}", tag=f"V{j}") for j in range(it)]
        VQ = [v_pool.tile([128, nh], F16, name=f"VQ{j}", tag=f"VQ{j}") for j in range(it)]

        # ---------------- projections ----------------
        with tc.tile_pool(name="proj", bufs=4) as stage_pool, \
             tc.tile_pool(name="wres", bufs=1) as w_pool, \
             tc.tile_pool(name="xres", bufs=1) as x_pool, \
             tc.tile_pool(name="ppsum", bufs=4, space="PSUM") as proj_psum:

            # stage + round x^T chunks to fp32r (resident)
            x_r = []
            for kc in range(KC):
                st = stage_pool.tile([128, seq], F32, tag="xstage")
                (nc.sync, nc.scalar, nc.gpsimd)[kc % 3].dma_start(
                    st[:], xT[128 * kc : 128 * (kc + 1), :]
                )
                xr = x_pool.tile([128, seq], F32R, tag=f"xr{kc}")
                eng = nc.vector if kc % 2 == 0 else nc.scalar
                if eng is nc.vector:
                    eng.tensor_copy(xr[:], st[:])
                else:
                    eng.copy(xr[:], st[:])
                x_r.append(xr)

            # stage + round weights
            w_r = {}
            for wi, (wname, wt) in enumerate([("q", Wq), ("k", Wk), ("v", Wv)]):
                wcols = hd + nh if wname == "v" else hd
                for kc in range(KC):
                    st = stage_pool.tile([128, wcols], F32, tag="wstage")
                    (nc.sync, nc.scalar, nc.gpsimd)[(wi * KC + kc) % 3].dma_start(
                        st[:], wt[128 * kc : 128 * (kc + 1), :]
                    )
                    wr = w_pool.tile([128, wcols], F32R, tag=f"w{wname}{kc}")
                    if (wi * KC + kc) % 2 == 0:
                        nc.vector.tensor_copy(wr[:], st[:])
                    else:
                        nc.scalar.copy(wr[:], st[:])
                    w_r[(wname, kc)] = wr

            # pos table: stage, round, duplicate
            st = stage_pool.tile([64, RW], F32, tag="tstage")
            nc.gpsimd.dma_start(st[:], TrevT[:])
            nc.vector.tensor_copy(trev_dup[0:64, :], st[:])
            nc.vector.tensor_copy(trev_dup[64:128, :], st[:])

            # q^T, k^T: out [hd, seq] as head-pair tiles [128, seq]
            ci = 0
            for wname, dest in [("q", qT), ("k", kT)]:
                for m in range(nh // 2):
                    for sh in range(seq // 512):
                        ps = proj_psum.tile([128, 512], F32, tag="pp", bufs=4)
                        for kc in range(KC):
                            nc.tensor.matmul(
                                ps[:],
                                w_r[(wname, kc)][:, 128 * m : 128 * (m + 1)],
                                x_r[kc][:, 512 * sh : 512 * (sh + 1)],
                                start=(kc == 0),
                                stop=(kc == KC - 1),
                            )
                        dst = dest[m][:, 512 * sh : 512 * (sh + 1)]
                        if ci % 2 == 0:
                            nc.vector.tensor_copy(dst, ps[:])
                        else:
                            nc.scalar.copy(dst, ps[:])
                        ci += 1

            # v: out [seq, hd] tiles -> V fp16; plus vq1024 [128, nh] (g=0 column)
            for jt in range(it):
                ps = proj_psum.tile([128, hd], F32, tag="vp", bufs=2)
                psq = proj_psum.tile([128, nh], F32, tag="vqp", bufs=2)
                for kc in range(KC):
                    nc.tensor.matmul(
                        ps[:],
                        x_r[kc][:, 128 * jt : 128 * (jt + 1)],
                        w_r[("v", kc)][:, 0:hd],
                        start=(kc == 0),
                        stop=(kc == KC - 1),
                    )
                    nc.tensor.matmul(
                        psq[:],
                        x_r[kc][:, 128 * jt : 128 * (jt + 1)],
                        w_r[("v", kc)][:, hd : hd + nh],
                        start=(kc == 0),
                        stop=(kc == KC - 1),
                    )
                if jt % 2 == 0:
                    nc.vector.tensor_copy(V[jt][:], ps[:])
                else:
                    nc.scalar.copy(V[jt][:], ps[:])
                nc.vector.tensor_copy(VQ[jt][:], psq[:])

        # ---------------- attention ----------------
        att = ctx.enter_context(tc.tile_pool(name="att", bufs=3))
        pt_pool = ctx.enter_context(tc.tile_pool(name="pt", bufs=1))
        ot_pool = ctx.enter_context(tc.tile_pool(name="ot", bufs=3))
        psum_s = ctx.enter_context(tc.tile_pool(name="psS", bufs=3, space="PSUM"))
        psum_qp = ctx.enter_context(tc.tile_pool(name="psQP", bufs=2, space="PSUM"))
        psum_o = ctx.enter_context(tc.tile_pool(name="psO", bufs=1, space="PSUM"))

        ptalls, ptvs = {}, {}

        exts = {}
        poss = {}
        pts = {}

        def emit_qp(h, t, ci):
            pair, base = h // 2, 64 * (h % 2)
            q_h = qT[pair][base : base + 64, :]
            t_h = trev_dup[base : base + 64, :]
            q_t = q_h[:, 128 * t : 128 * (t + 1)]

            if "qp" in ablate:
                assert "posdma" in ablate
                return
            ext = att.tile([128, EXT], F16, tag="ext", name=f"ext{h}_{t}", bufs=6)
            if True:
                for qi, (c0, c1) in enumerate(((0, 512), (512, 1024))):
                    ps_qp = psum_qp.tile(
                        [128, 512], F32, tag="QP", name=f"qp{h}_{t}_{qi}", bufs=4
                    )
                    nc.tensor.matmul(ps_qp[:], q_t, t_h[:, c0:c1], start=True, stop=True)
                    if (2 * ci + qi) % 2 == 0:
                        nc.vector.tensor_copy(ext[:, 511 + c0 : 511 + c1], ps_qp[:])
                    else:
                        nc.scalar.copy(ext[:, 511 + c0 : 511 + c1], ps_qp[:])
                nc.gpsimd.tensor_copy(ext[:, 1535:1536], VQ[t][:, h : h + 1])
            lw = max(0, 511 - (seq - 128 - 128 * t))
            rw_ = min(max(0, (seq - 1 - 128 * t) + seq - 1 - 1535), EXT - 1536)
            if "qp" in ablate or "fills" in ablate:
                lw = rw_ = 0
            if lw > 0:
                nc.gpsimd.tensor_copy(
                    ext[:, 511 - lw : 511],
                    ext[:, 511:512].to_broadcast([128, lw]),
                )
            if rw_ > 0:
                nc.gpsimd.tensor_copy(
                    ext[:, 1536 : 1536 + rw_],
                    ext[:, 1535:1536].to_broadcast([128, rw_]),
                )

            pos = att.tile([128, seq], F16, tag="pos", name=f"pos{h}_{t}", bufs=14)
            poss[(h, t)] = pos
            extf = ext[:]
            if "posdma" in ablate:
                nc.gpsimd.memset(pos[:], 0.01)
            elif "skew" in ablate:
                nc.sync.dma_start(pos[:], ext[:, 0:seq])
            else:
                diag = AP(
                    tensor=extf.tensor,
                    offset=extf.offset + (seq - 1 - 128 * t),
                    ap=[[EXT - 1, 128], [1, seq]],
                )
                eng = nc.sync
                eng.dma_start(pos[:], diag)

        def emit_tile(h, t):
            pair, base = h // 2, 64 * (h % 2)
            q_h = qT[pair][base : base + 64, :]
            k_h = kT[pair][base : base + 64, :]
            ptv = ptvs[h]
            q_t = q_h[:, 128 * t : 128 * (t + 1)]
            pos = poss.pop((h, t), None)

            s_sb = att.tile([128, seq], F32, tag="ssb", name=f"ssb{h}_{t}", bufs=4)
            for jh in range(seq // 512):
                sl = slice(512 * jh, 512 * (jh + 1))
                ps_s = psum_s.tile([128, 512], F32, tag="S", name=f"s{h}_{t}_{jh}", bufs=3)
                nc.tensor.matmul(ps_s[:], q_t, k_h[:, sl], start=True, stop=True)
                if "add" not in ablate and pos is not None:
                    nc.vector.tensor_add(s_sb[:, sl], ps_s[:], pos[:, sl])
                else:
                    nc.vector.tensor_copy(s_sb[:, sl], ps_s[:])

            p_t = att.tile([128, seq], F16, tag="pt", name=f"p{h}_{t}", bufs=10)
            z_t = att.tile([128, 1], F32, tag="zt", name=f"z{h}_{t}", bufs=10)
            if "exp" not in ablate:
                nc.scalar.activation(
                    p_t[:], s_sb[:], mybir.ActivationFunctionType.Exp,
                    bias=exp_bias[:], scale=1.0, accum_out=z_t[:],
                )
            else:
                nc.vector.tensor_copy(p_t[:], s_sb[:])
                nc.gpsimd.memset(z_t[:], 1.0)
            pts[(h, t)] = (p_t, z_t)

        def emit_norm(h, t):
            ptv = ptvs[h]  # [p, t, x=di*128+m]
            p_t, z_t = pts.pop((h, t))
            zr_t = att.tile([128, 1], F32, tag="zrt", name=f"zr{h}_{t}")
            nc.vector.reciprocal(zr_t[:], z_t[:])
            nc.vector.tensor_scalar_mul(p_t[:], p_t[:], zr_t[:])
            dst = ptv[:, :, 128 * t : 128 * (t + 1)]
            teng = nc.scalar
            if "transpose" not in ablate:
                teng.dma_start_transpose(out=dst, in_=p_t[:])
            else:
                teng.dma_start(dst, p_t[:].rearrange("p (a b) -> p a b", a=it))

        ots = {}

        def emit_pv_part(h, ih):
            ptv = ptvs[h]
            if ih == 0:
                ots[h] = ot_pool.tile([D, seq], F32, tag="ot", name=f"ot{h}", bufs=4)
            ot = ots[h]
            ps_o = psum_o.tile([D, 512], F32, tag="O", name=f"o{h}_{ih}")
            for jb in range(it):
                nc.tensor.matmul(
                    ps_o[:],
                    V[jb][:, D * h : D * (h + 1)],
                    ptv[:, jb, 512 * ih : 512 * (ih + 1)],
                    start=(jb == 0),
                    stop=(jb == it - 1),
                )
            if ih % 2 == 0:
                nc.vector.tensor_copy(ot[:, 512 * ih : 512 * (ih + 1)], ps_o[:])
            else:
                nc.scalar.copy(ot[:, 512 * ih : 512 * (ih + 1)], ps_o[:])
            if ih == seq // 512 - 1:
                nc.sync.dma_start(out[h], ots.pop(h)[:])

        def emit_pv(h):
            for ih in range(seq // 512):
                emit_pv_part(h, ih)

        ci = 0
        for h in range(nh):
            ptalls[h] = pt_pool.tile(
                [128, it * seq], F16, tag=f"ptall{h % 2}", name=f"ptall{h}"
            )
            ptvs[h] = ptalls[h].rearrange("p (j i) -> p j i", j=it)
        for t in range(it):
            emit_qp(0, t, ci); ci += 1
        for h in range(nh):
            for t in range(it):
                if h + 1 < nh:
                    emit_qp(h + 1, t, ci); ci += 1
                emit_tile(h, t)
            for t in range(it):
                emit_norm(h, t)
            if h > 0:
                emit_pv(h - 1)
        emit_pv(nh - 1)

    nc.compile()
    return nc


def prep_inputs(x, Wq, Wkv, pos_emb):
    """Host-side shard prep: returns list of 8 per-core input dicts."""
    x = np.asarray(x, dtype=np.float32)
    Wq = np.asarray(Wq, dtype=np.float32)
    Wkv = np.asarray(Wkv, dtype=np.float32)
    pos_emb = np.asarray(pos_emb, dtype=np.float32)

    Wq_s = (Wq * SCALE).astype(np.float32)
    trevT = np.ascontiguousarray(pos_emb[::-1].T[:, :RW])    # [64, 1024], col c = T[1024-c]

    in_maps = []
    for c in range(NC):
        b, hg = c // 2, c % 2
        hs = slice(HD * hg, HD * (hg + 1))
        wq_c = np.ascontiguousarray(Wq_s[:, hs])
        # W*: per-head derived column so that x @ W*_h = q_h . T[0] (dist g=0)
        wstar = np.einsum(
            "dhe,e->dh", wq_c.reshape(DIM, NH, D), pos_emb[0].astype(np.float32)
        )
        in_maps.append(
            {
                "xT": np.ascontiguousarray(x[b].T),
                "Wq": wq_c,
                "Wk": np.ascontiguousarray(Wkv[:, hs]),
                "Wv": np.ascontiguousarray(
                    np.concatenate([Wkv[:, DIM:][:, hs], wstar], axis=1)
                ),
                "TrevT": trevT,
            }
        )
    return in_maps


def assemble(results):
    """results: list of 8 out maps (each {'out': [8, 64, 1024]}) -> [4,16,1024,64]."""
    out = np.empty((B, N_HEADS_TOT, SEQ, D), dtype=np.float32)
    for c in range(NC):
        b, hg = c // 2, c % 2
        out[b, NH * hg : NH * (hg + 1)] = np.transpose(results[c]["out"], (0, 2, 1))
    return out


def kernel(x, Wq, Wkv, pos_emb, trace=False, trace_kwargs=None, bench_iters=1, ablate=()):
    key = ("nc", bench_iters, tuple(sorted(ablate)))
    if key not in _cached:
        _cached[key] = build_nc(bench_iters=bench_iters, ablate=ablate)
    nc = _cached[key]
    in_maps = prep_inputs(x, Wq, Wkv, pos_emb)
    res = run_bass_kernel_spmd(
        nc, in_maps, list(range(NC)), trace=trace, **(trace_kwargs or {})
    )
    out = assemble(res.results)
    if trace:
        _cached["last_result"] = res
    return out


if __name__ == "__main__":
    # smoke: random check against numpy reference
    rng = np.random.default_rng(0)
    x = rng.standard_normal((B, SEQ, DIM), dtype=np.float32)
    Wq = (rng.standard_normal((DIM, 1024), dtype=np.float32) * DIM ** -0.5)
    Wkv = (rng.standard_normal((DIM, 2048), dtype=np.float32) * DIM ** -0.5)
    pos_emb = rng.standard_normal((TABLE, D), dtype=np.float32)
    out = kernel(x, Wq, Wkv, pos_emb)
    print("out shape", out.shape, "finite:", np.isfinite(out).all())

